# revision 5
# baseline (speedup 1.0000x reference)
"""Batch-parallel flash attention for [16, 4096, 128] f32 on 8 TRN2 NeuronCores.

Sharding: data-parallel over batch. Core c gets batches [2c, 2c+2) and computes
full attention for them. No collectives.

Per-core algorithm (per batch, transposed-score layout):
  - Load Q, K natural [n,128] tiles, PE-transpose to Q^T, K^T [128d, n] (bf16).
  - Load V natural [k,128] tiles (bf16).
  - For each 512-wide q chunk:
      S^T[k, q] = matmul(lhsT=K^T tile, rhs=Q^T chunk)   (PSUM f32)
      P^T = exp(S^T * 1/sqrt(128))                        (ScalarE -> SBUF bf16)
      out^T[v, q] += matmul(lhsT=V tile, rhs=P^T tile)    (PSUM f32, 32 kc)
      denom[q] = partition_all_reduce(tree_sum_kc(P^T))   (GpSimd+DVE)
      out^T *= 1/denom  (DVE, fused into PSUM->SBUF copy) -> DMA out transposed
  - Host transposes [b, v, q] -> [b, q, v] during unshard.
"""

import os
import sys

for _p in ("/opt/trn_rl_repo", "/root/.axon_site/_ro/trn_rl_repo"):
    if os.path.isdir(_p) and _p not in sys.path:
        sys.path.insert(0, _p)

import numpy as np
from contextlib import ExitStack

import concourse.bass as bass
import concourse.tile as tile
from concourse import mybir
from concourse.bass_utils import run_bass_kernel_spmd
from concourse.masks import make_identity


def _split_waits(nc, cap=1):
    """Walrus in this toolchain rejects instructions with >cap sync waits.

    Hoist excess waits into standalone EventSemaphore instructions on the
    same engine, immediately before the original instruction (semantics
    preserved: same engine stream, same order, waits still all precede
    execution)."""
    cnt = 0
    for f in nc.m.functions:
        for blk in f.blocks:
            new = []
            for inst in blk.instructions:
                si = inst.sync_info
                if si is not None and len(si.on_wait) > cap:
                    waits = list(si.on_wait)
                    excess, keep = waits[:-cap], waits[-cap:]
                    for w in excess:
                        cnt += 1
                        wi = mybir.InstEventSemaphore(
                            name=f"wsplit-{cnt}", ins=[], outs=[],
                            sync_info=mybir.SyncInfo(on_wait=[w], on_update=[]),
                        )
                        wi.engine = inst.engine
                        new.append(wi)
                    inst.sync_info = mybir.SyncInfo(
                        on_wait=keep, on_update=list(si.on_update)
                    )
                new.append(inst)
            blk.instructions = new
    return cnt

B, N, D = 16, 4096, 128
NCORES = 8
BPC = B // NCORES  # 2 batches per core
P = 128
QC = 512            # q columns per chunk
NQC = N // QC       # 8
NKC = N // P        # 32 key chunks
SCALE = float(1.0 / np.sqrt(np.float32(D)))

F32 = mybir.dt.float32
BF16 = mybir.dt.bfloat16

# key-chunks per exp group; groups of 3 (plus one of 2) keep the PSUM budget at
# 2*3 (scores, double-buffered) + 2*1 (out^T) = 8 banks.
GROUPS = [3, 3, 3, 3, 3, 3, 3, 3, 3, 3, 2]
assert sum(GROUPS) == NKC


def _build():
    nc = bass.Bass(target_bir_lowering=False, trn_type="TRN2")

    q_ext = nc.declare_dram_parameter("query", [BPC, N, D], F32, isOutput=False)
    k_ext = nc.declare_dram_parameter("key", [BPC, N, D], F32, isOutput=False)
    v_ext = nc.declare_dram_parameter("value", [BPC, N, D], F32, isOutput=False)
    # transposed output: [b, v, q]
    out_ext = nc.declare_dram_parameter("out", [BPC, D, N], F32, isOutput=True)

    with ExitStack() as ctx:
        tc = ctx.enter_context(tile.TileContext(nc))

        const = ctx.enter_context(tc.tile_pool(name="const", bufs=1))
        nat = ctx.enter_context(tc.tile_pool(name="nat", bufs=8))
        qt_pool = ctx.enter_context(tc.tile_pool(name="qt", bufs=2 * NQC))
        kt_pool = ctx.enter_context(tc.tile_pool(name="kt", bufs=2 * NQC))
        vn_pool = ctx.enter_context(tc.tile_pool(name="vn", bufs=2 * NKC))
        pt_pool = ctx.enter_context(tc.tile_pool(name="pt", bufs=2 * len(GROUPS) + 2))
        tree_pool = ctx.enter_context(tc.tile_pool(name="tree", bufs=24))
        r_pool = ctx.enter_context(tc.tile_pool(name="r", bufs=3))
        ot_pool = ctx.enter_context(tc.tile_pool(name="ot", bufs=3))
        psum_s = ctx.enter_context(tc.tile_pool(name="psum_s", bufs=2, space="PSUM"))
        psum_o = ctx.enter_context(tc.tile_pool(name="psum_o", bufs=2, space="PSUM"))

        ident = const.tile([P, P], F32)
        make_identity(nc, ident)
        ones_col = const.tile([P, 1], BF16)   # lhsT for partition-sum matmul
        nc.vector.memset(ones_col, 1.0)
        ones_row = const.tile([1, P], F32)    # lhsT for broadcast outer-product
        nc.vector.memset(ones_row, 1.0)

        for b in range(BPC):
            # ---------------- prologue: load + transpose Q, K; load V ----------
            qt_tiles = []
            kt_tiles = []
            for src_ext, tiles, tag in ((k_ext, kt_tiles, "kt"), (q_ext, qt_tiles, "qt")):
                pool = kt_pool if tag == "kt" else qt_pool
                for t in range(NQC):  # 512-wide n groups
                    ps = psum_o.tile([P, QC], F32, tag="o")
                    for j in range(4):
                        nchunk = t * 4 + j
                        natt = nat.tile([P, P], F32, tag="nat")
                        nc.sync.dma_start(
                            out=natt, in_=src_ext[b, nchunk * P:(nchunk + 1) * P, :]
                        )
                        nc.tensor.transpose(ps[:, j * P:(j + 1) * P], natt, ident)
                    dst = pool.tile([P, QC], BF16, tag=tag)
                    nc.vector.tensor_copy(out=dst, in_=ps)  # f32 -> bf16
                    tiles.append(dst)

            vn_tiles = []
            for kc in range(NKC):
                natt = nat.tile([P, P], F32, tag="nat")
                nc.sync.dma_start(out=natt, in_=v_ext[b, kc * P:(kc + 1) * P, :])
                vt = vn_pool.tile([P, P], BF16, tag="vn")
                nc.gpsimd.tensor_copy(out=vt, in_=natt)
                vn_tiles.append(vt)

            # ---------------- main loop over q chunks --------------------------
            for qc in range(NQC):
                rhs_q = qt_tiles[qc]
                psum_out = psum_o.tile([P, QC], F32, tag="o")
                pt_slices = []  # 32 [P, QC] bf16 APs, one per kc
                pv_emitted = 0

                def emit_pv(upto):
                    nonlocal pv_emitted
                    while pv_emitted < upto:
                        kc = pv_emitted
                        nc.tensor.matmul(
                            psum_out,
                            lhsT=vn_tiles[kc],
                            rhs=pt_slices[kc],
                            start=(kc == 0),
                            stop=(kc == NKC - 1),
                        )
                        pv_emitted += 1

                kc0 = 0
                for gi, gsz in enumerate(GROUPS):
                    ps = psum_s.tile([P, 3 * QC], F32, tag="s")
                    for j in range(gsz):
                        kc = kc0 + j
                        nc.tensor.matmul(
                            ps[:, j * QC:(j + 1) * QC],
                            lhsT=kt_tiles[kc // 4][:, (kc % 4) * P:(kc % 4 + 1) * P],
                            rhs=rhs_q,
                            start=True,
                            stop=True,
                        )
                    pt = pt_pool.tile([P, 3 * QC], BF16, tag="pt")
                    nc.scalar.activation(
                        out=pt[:, : gsz * QC],
                        in_=ps[:, : gsz * QC],
                        func=mybir.ActivationFunctionType.Exp,
                        scale=SCALE,
                    )
                    for j in range(gsz):
                        pt_slices.append(pt[:, j * QC:(j + 1) * QC])
                    kc0 += gsz
                    # overlap PV of previous group with next group's scores
                    emit_pv(len(pt_slices) - gsz if gi > 0 else 0)
                emit_pv(NKC)

                # ---- softmax denominator: tree-sum the 32 P^T slices over kc
                cur = pt_slices
                lvl = 0
                while len(cur) > 1:
                    nxt = []
                    for i in range(0, len(cur) - 1, 2):
                        t = tree_pool.tile([P, QC], BF16, tag="tree")
                        eng = nc.gpsimd if lvl == 0 else nc.vector
                        eng.tensor_add(out=t, in0=cur[i], in1=cur[i + 1])
                        nxt.append(t)
                    if len(cur) % 2:
                        nxt.append(cur[-1])
                    cur = nxt
                    lvl += 1
                acc = cur[0]

                # partition-sum via ones-matmul: denom[1, q] = sum_p acc[p, q]
                psum_d = psum_o.tile([1, QC], F32, tag="o")
                nc.tensor.matmul(psum_d, lhsT=ones_col, rhs=acc, start=True, stop=True)
                rec1 = r_pool.tile([1, QC], F32, tag="rec1")
                nc.vector.reciprocal(rec1, psum_d)
                # broadcast across partitions via outer product ones[128,1] @ rec1[1,q]
                psum_r = psum_o.tile([P, QC], F32, tag="o")
                nc.tensor.matmul(psum_r, lhsT=ones_row, rhs=rec1, start=True, stop=True)
                rbc = r_pool.tile([P, QC], F32, tag="rbc")
                nc.vector.tensor_copy(out=rbc, in_=psum_r)

                # ---- normalize + store (transposed layout [v, q])
                ot = ot_pool.tile([P, QC], F32, tag="ot")
                nc.vector.tensor_mul(out=ot, in0=psum_out, in1=rbc)
                nc.sync.dma_start(
                    out=out_ext[b, :, qc * QC:(qc + 1) * QC], in_=ot
                )

    return nc


_NC_CACHE = None


def _get_nc():
    global _NC_CACHE
    if _NC_CACHE is None:
        nc = _build()
        _split_waits(nc)
        _NC_CACHE = nc
    return _NC_CACHE


LAST_RESULT = None


def kernel(query, key, value):
    q = np.ascontiguousarray(np.asarray(query), dtype=np.float32)
    k = np.ascontiguousarray(np.asarray(key), dtype=np.float32)
    v = np.ascontiguousarray(np.asarray(value), dtype=np.float32)
    assert q.shape == (B, N, D), q.shape

    nc = _get_nc()
    in_maps = []
    for c in range(NCORES):
        sl = slice(c * BPC, (c + 1) * BPC)
        in_maps.append({
            "query": np.ascontiguousarray(q[sl]),
            "key": np.ascontiguousarray(k[sl]),
            "value": np.ascontiguousarray(v[sl]),
        })

    global LAST_RESULT
    res = run_bass_kernel_spmd(nc, in_maps, core_ids=list(range(NCORES)))
    LAST_RESULT = res

    parts = []
    for c in range(NCORES):
        o = np.asarray(res.results[c]["out"])  # [BPC, D, N]
        parts.append(o.transpose(0, 2, 1))     # -> [BPC, N, D]
    return np.ascontiguousarray(np.concatenate(parts, axis=0), dtype=np.float32)


if __name__ == "__main__":
    nc = _get_nc()
    print("built ok")


# revision 7
# speedup vs baseline: 1.2103x; 1.2103x over previous
"""Batch-parallel flash attention for [16, 4096, 128] f32 on 8 TRN2 NeuronCores.

Sharding: data-parallel over batch. Core c gets batches [2c, 2c+2) and computes
full attention for them. No collectives.

Per-core algorithm (per batch, transposed-score layout):
  - Load Q, K natural [n,128] f32 tiles, PE-transpose + cast -> Q^T, K^T
    [128d, n] bf16 in SBUF. Load V natural, cast to bf16.
  - For each 512-wide q chunk:
      S^T[k, q] = matmul(lhsT=K^T tile, rhs=Q^T chunk)   (PSUM f32, KG=2 kc/group)
      P^T = exp(S^T * 1/sqrt(128))                        (ScalarE -> SBUF bf16)
      out^T[v, q] += matmul(lhsT=V tile, rhs=P^T tile)    (PSUM f32, 32 kc)
      denom[q]: kc 0..PE_KC-1 summed by accumulating ones-matmuls on PE;
                kc PE_KC..31 by pairwise adds on GpSimd/DVE, folded in by one
                final ones-matmul.  1/denom via DVE fast reciprocal, broadcast
                across partitions by a PE outer-product, multiply on DVE.
  - Device emits out^T [b, v, q]; host transposes to [b, q, v] during unshard.

PSUM budget: scores 2x2 banks + out^T 2 + misc(d/r/input-transpose) 2 = 8.
"""

import os
import sys

for _p in ("/opt/trn_rl_repo", "/root/.axon_site/_ro/trn_rl_repo"):
    if os.path.isdir(_p) and _p not in sys.path:
        sys.path.insert(0, _p)

import numpy as np
from contextlib import ExitStack

import concourse.bass as bass
import concourse.tile as tile
from concourse import mybir
from concourse.bass_utils import run_bass_kernel_spmd
from concourse.masks import make_identity

B, N, D = 16, 4096, 128
NCORES = 8
BPC = B // NCORES  # 2 batches per core
P = 128
QC = 512            # q columns per chunk
NQC = N // QC       # 8 chunks per batch
NKC = N // P        # 32 key chunks
KG = 2              # kc per score/exp group
NG = NKC // KG      # 16 groups
PE_KC = 8           # kc whose denom contribution is summed on PE (rest on DVE/GpSimd)
SCALE = float(1.0 / np.sqrt(np.float32(D)))

F32 = mybir.dt.float32
BF16 = mybir.dt.bfloat16

assert PE_KC % KG == 0


def _split_waits(nc, cap=1):
    """Walrus in this toolchain rejects instructions with >cap sync waits.

    Hoist excess waits into standalone EventSemaphore instructions on the
    same engine, immediately before the original instruction (semantics
    preserved: same engine stream, same order, waits still all precede
    execution)."""
    cnt = 0
    for f in nc.m.functions:
        for blk in f.blocks:
            new = []
            for inst in blk.instructions:
                si = inst.sync_info
                if si is not None and len(si.on_wait) > cap:
                    waits = list(si.on_wait)
                    excess, keep = waits[:-cap], waits[-cap:]
                    for w in excess:
                        cnt += 1
                        wi = mybir.InstEventSemaphore(
                            name=f"wsplit-{cnt}", ins=[], outs=[],
                            sync_info=mybir.SyncInfo(on_wait=[w], on_update=[]),
                        )
                        wi.engine = inst.engine
                        new.append(wi)
                    inst.sync_info = mybir.SyncInfo(
                        on_wait=keep, on_update=list(si.on_update)
                    )
                new.append(inst)
            blk.instructions = new
    return cnt


def _build():
    nc = bass.Bass(target_bir_lowering=False, trn_type="TRN2")

    q_ext = nc.declare_dram_parameter("query", [BPC, N, D], F32, isOutput=False)
    k_ext = nc.declare_dram_parameter("key", [BPC, N, D], F32, isOutput=False)
    v_ext = nc.declare_dram_parameter("value", [BPC, N, D], F32, isOutput=False)
    out_ext = nc.declare_dram_parameter("out", [BPC, D, N], F32, isOutput=True)

    with ExitStack() as ctx:
        tc = ctx.enter_context(tile.TileContext(nc))

        const = ctx.enter_context(tc.tile_pool(name="const", bufs=1))
        nat = ctx.enter_context(tc.tile_pool(name="nat", bufs=8))
        stage = ctx.enter_context(tc.tile_pool(name="stage", bufs=4))
        qt_pool = ctx.enter_context(tc.tile_pool(name="qt", bufs=2 * NQC))
        kt_pool = ctx.enter_context(tc.tile_pool(name="kt", bufs=2 * NQC))
        vn_pool = ctx.enter_context(tc.tile_pool(name="vn", bufs=2 * NQC))
        pt_pool = ctx.enter_context(tc.tile_pool(name="pt", bufs=20))
        tree_pool = ctx.enter_context(tc.tile_pool(name="tree", bufs=16))
        r_pool = ctx.enter_context(tc.tile_pool(name="r", bufs=3))
        ot_pool = ctx.enter_context(tc.tile_pool(name="ot", bufs=3))
        psum_s = ctx.enter_context(tc.tile_pool(name="psum_s", bufs=2, space="PSUM"))
        psum_out = ctx.enter_context(tc.tile_pool(name="psum_out", bufs=2, space="PSUM"))
        psum_misc = ctx.enter_context(tc.tile_pool(name="psum_misc", bufs=2, space="PSUM"))

        ident = const.tile([P, P], F32)
        make_identity(nc, ident)
        ones_col = const.tile([P, 1], BF16)   # lhsT for partition-sum matmuls
        nc.vector.memset(ones_col, 1.0)
        ones_row = const.tile([1, P], F32)    # lhsT for broadcast outer-product
        nc.vector.memset(ones_row, 1.0)

        for b in range(BPC):
            # ------------- prologue: load + transpose Q,K; load + cast V ------
            qt_tiles = []   # 8 tiles [P, QC] bf16: Q^T  (d on partitions)
            kt_tiles = []   # 8 tiles [P, QC] bf16: K^T
            vn_tiles = []   # 8 tiles [P, QC] bf16: V natural, 4 kc each
            for t in range(NQC):
                for src_ext, tiles, pool, tag in (
                    (k_ext, kt_tiles, kt_pool, "kt"),
                    (q_ext, qt_tiles, qt_pool, "qt"),
                ):
                    ps = psum_misc.tile([P, QC], F32, tag="misc")
                    for j in range(4):
                        nchunk = t * 4 + j
                        natt = nat.tile([P, P], F32, tag="nat")
                        nc.sync.dma_start(
                            out=natt, in_=src_ext[b, nchunk * P:(nchunk + 1) * P, :]
                        )
                        nc.tensor.transpose(ps[:, j * P:(j + 1) * P], natt, ident)
                    dst = pool.tile([P, QC], BF16, tag=tag)
                    nc.vector.tensor_copy(out=dst, in_=ps)  # f32 -> bf16
                    tiles.append(dst)
                vst = stage.tile([P, QC], F32, tag="vst")
                for j in range(4):
                    nchunk = t * 4 + j
                    nc.sync.dma_start(
                        out=vst[:, j * P:(j + 1) * P],
                        in_=v_ext[b, nchunk * P:(nchunk + 1) * P, :],
                    )
                vt = vn_pool.tile([P, QC], BF16, tag="vn")
                nc.vector.tensor_copy(out=vt, in_=vst)
                vn_tiles.append(vt)

            def kt_slice(kc):
                return kt_tiles[kc // 4][:, (kc % 4) * P:(kc % 4 + 1) * P]

            def vn_slice(kc):
                return vn_tiles[kc // 4][:, (kc % 4) * P:(kc % 4 + 1) * P]

            # ------------- main loop over q chunks ----------------------------
            for qc in range(NQC):
                rhs_q = qt_tiles[qc]
                pout = psum_out.tile([P, QC], F32, tag="out")
                pt_slices = [None] * NKC
                leaves = []  # tree leaves for kc >= PE_KC

                for g in range(NG):
                    ps = psum_s.tile([P, KG * QC], F32, tag="s")
                    for j in range(KG):
                        kc = g * KG + j
                        nc.tensor.matmul(
                            ps[:, j * QC:(j + 1) * QC],
                            lhsT=kt_slice(kc),
                            rhs=rhs_q,
                            start=True,
                            stop=True,
                        )
                    pt = pt_pool.tile([P, KG * QC], BF16, tag="pt")
                    nc.scalar.activation(
                        out=pt, in_=ps,
                        func=mybir.ActivationFunctionType.Exp,
                        scale=SCALE,
                    )
                    for j in range(KG):
                        pt_slices[g * KG + j] = pt[:, j * QC:(j + 1) * QC]
                    # PV for the previous group keeps PE dense while ACT works
                    if g >= 1:
                        for kc in range((g - 1) * KG, g * KG):
                            nc.tensor.matmul(
                                pout, lhsT=vn_slice(kc), rhs=pt_slices[kc],
                                start=(kc == 0), stop=(kc == NKC - 1),
                            )
                    # denominator leaves on GpSimd for the tree range
                    if g * KG >= PE_KC:
                        t = tree_pool.tile([P, QC], BF16, tag="tree")
                        nc.gpsimd.tensor_add(
                            out=t, in0=pt[:, 0:QC], in1=pt[:, QC:2 * QC]
                        )
                        leaves.append(t)
                for kc in range((NG - 1) * KG, NKC):
                    nc.tensor.matmul(
                        pout, lhsT=vn_slice(kc), rhs=pt_slices[kc],
                        start=(kc == 0), stop=(kc == NKC - 1),
                    )

                # ---- denominator: PE part (kc < PE_KC) accumulates in psum_d
                psum_d = psum_misc.tile([1, QC], F32, tag="misc")
                for kc in range(PE_KC):
                    nc.tensor.matmul(
                        psum_d, lhsT=ones_col, rhs=pt_slices[kc],
                        start=(kc == 0), stop=False, skip_group_check=True,
                    )
                # ---- tree part on DVE
                cur = leaves
                while len(cur) > 1:
                    nxt = []
                    for i in range(0, len(cur) - 1, 2):
                        t = tree_pool.tile([P, QC], BF16, tag="tree")
                        nc.vector.tensor_add(out=t, in0=cur[i], in1=cur[i + 1])
                        nxt.append(t)
                    if len(cur) % 2:
                        nxt.append(cur[-1])
                    cur = nxt
                acc = cur[0]
                nc.tensor.matmul(
                    psum_d, lhsT=ones_col, rhs=acc,
                    start=False, stop=True, skip_group_check=True,
                )

                rec1 = r_pool.tile([1, QC], F32, tag="rec1")
                nc.vector.reciprocal(rec1, psum_d)
                psum_r = psum_misc.tile([P, QC], F32, tag="misc")
                nc.tensor.matmul(
                    psum_r, lhsT=ones_row, rhs=rec1, start=True, stop=True
                )
                rbc = r_pool.tile([P, QC], F32, tag="rbc")
                nc.vector.tensor_copy(out=rbc, in_=psum_r)

                # ---- normalize + store (transposed layout [v, q])
                ot = ot_pool.tile([P, QC], F32, tag="ot")
                nc.vector.tensor_mul(out=ot, in0=pout, in1=rbc)
                nc.sync.dma_start(
                    out=out_ext[b, :, qc * QC:(qc + 1) * QC], in_=ot
                )

    return nc


_NC_CACHE = None


def _get_nc():
    global _NC_CACHE
    if _NC_CACHE is None:
        nc = _build()
        _split_waits(nc)
        _NC_CACHE = nc
    return _NC_CACHE


LAST_RESULT = None


def kernel(query, key, value):
    q = np.ascontiguousarray(np.asarray(query), dtype=np.float32)
    k = np.ascontiguousarray(np.asarray(key), dtype=np.float32)
    v = np.ascontiguousarray(np.asarray(value), dtype=np.float32)
    assert q.shape == (B, N, D), q.shape

    nc = _get_nc()
    in_maps = []
    for c in range(NCORES):
        sl = slice(c * BPC, (c + 1) * BPC)
        in_maps.append({
            "query": np.ascontiguousarray(q[sl]),
            "key": np.ascontiguousarray(k[sl]),
            "value": np.ascontiguousarray(v[sl]),
        })

    global LAST_RESULT
    res = run_bass_kernel_spmd(nc, in_maps, core_ids=list(range(NCORES)))
    LAST_RESULT = res

    parts = []
    for c in range(NCORES):
        o = np.asarray(res.results[c]["out"])  # [BPC, D, N]
        parts.append(o.transpose(0, 2, 1))     # -> [BPC, N, D]
    return np.ascontiguousarray(np.concatenate(parts, axis=0), dtype=np.float32)


if __name__ == "__main__":
    nc = _get_nc()
    print("built ok")


# revision 9
# speedup vs baseline: 1.3220x; 1.0923x over previous
"""Batch-parallel flash attention for [16, 4096, 128] f32 on 8 TRN2 NeuronCores.

Sharding: data-parallel over batch. Core c gets batches [2c, 2c+2) and computes
full attention for them. No collectives.

Per-core algorithm (per batch, transposed-score layout):
  - Load Q, K natural [n,128] f32 tiles, PE-transpose + cast -> Q^T, K^T
    [128d, n] bf16 in SBUF. Load V natural, cast to bf16.
  - For each 512-wide q chunk:
      S^T[k, q] = matmul(lhsT=K^T tile, rhs=Q^T chunk)   (PSUM f32, KG=2 kc/group)
      P^T = exp(S^T * 1/sqrt(128))                        (ScalarE -> SBUF bf16)
      out^T[v, q] += matmul(lhsT=V tile, rhs=P^T tile)    (PSUM f32, 32 kc)
      denom[q]: kc 0..PE_KC-1 summed by accumulating ones-matmuls on PE;
                kc PE_KC..31 by pairwise adds on GpSimd/DVE, folded in by one
                final ones-matmul.  1/denom via DVE fast reciprocal, broadcast
                across partitions by a PE outer-product, multiply on DVE.
  - Device emits out^T [b, v, q]; host transposes to [b, q, v] during unshard.

PSUM budget: scores 2x2 banks + out^T 2 + misc(d/r/input-transpose) 2 = 8.
"""

import os
import sys

for _p in ("/opt/trn_rl_repo", "/root/.axon_site/_ro/trn_rl_repo"):
    if os.path.isdir(_p) and _p not in sys.path:
        sys.path.insert(0, _p)

import numpy as np
from contextlib import ExitStack

import concourse.bass as bass
import concourse.tile as tile
from concourse import mybir
from concourse.bass_utils import run_bass_kernel_spmd
from concourse.masks import make_identity

B, N, D = 16, 4096, 128
NCORES = 8
BPC = B // NCORES  # 2 batches per core
P = 128
QC = 512            # q columns per chunk
NQC = N // QC       # 8 chunks per batch
NKC = N // P        # 32 key chunks
KG = 2              # kc per score/exp group
NG = NKC // KG      # 16 groups
PE_KC = 8           # kc whose denom contribution is summed on PE (rest on DVE/GpSimd)
SCALE = float(1.0 / np.sqrt(np.float32(D)))

F32 = mybir.dt.float32
BF16 = mybir.dt.bfloat16

assert PE_KC % KG == 0


def _split_waits(nc, cap=1):
    """Walrus in this toolchain rejects instructions with >cap sync waits.

    Hoist excess waits into standalone EventSemaphore instructions on the
    same engine, immediately before the original instruction (semantics
    preserved: same engine stream, same order, waits still all precede
    execution)."""
    cnt = 0
    for f in nc.m.functions:
        for blk in f.blocks:
            new = []
            for inst in blk.instructions:
                si = inst.sync_info
                if si is not None and len(si.on_wait) > cap:
                    waits = list(si.on_wait)
                    excess, keep = waits[:-cap], waits[-cap:]
                    for w in excess:
                        cnt += 1
                        wi = mybir.InstEventSemaphore(
                            name=f"wsplit-{cnt}", ins=[], outs=[],
                            sync_info=mybir.SyncInfo(on_wait=[w], on_update=[]),
                        )
                        wi.engine = inst.engine
                        new.append(wi)
                    inst.sync_info = mybir.SyncInfo(
                        on_wait=keep, on_update=list(si.on_update)
                    )
                new.append(inst)
            blk.instructions = new
    return cnt


def _build():
    nc = bass.Bass(target_bir_lowering=False, trn_type="TRN2")

    q_ext = nc.declare_dram_parameter("query", [BPC, N, D], F32, isOutput=False)
    k_ext = nc.declare_dram_parameter("key", [BPC, N, D], F32, isOutput=False)
    v_ext = nc.declare_dram_parameter("value", [BPC, N, D], F32, isOutput=False)
    out_ext = nc.declare_dram_parameter("out", [BPC, D, N], F32, isOutput=True)

    with ExitStack() as ctx:
        tc = ctx.enter_context(tile.TileContext(nc))

        const = ctx.enter_context(tc.tile_pool(name="const", bufs=1))
        nat = ctx.enter_context(tc.tile_pool(name="nat", bufs=8))
        stage = ctx.enter_context(tc.tile_pool(name="stage", bufs=4))
        qt_pool = ctx.enter_context(tc.tile_pool(name="qt", bufs=2 * NQC))
        kt_pool = ctx.enter_context(tc.tile_pool(name="kt", bufs=2 * NQC))
        vn_pool = ctx.enter_context(tc.tile_pool(name="vn", bufs=2 * NQC))
        pt_pool = ctx.enter_context(tc.tile_pool(name="pt", bufs=26))
        tree_pool = ctx.enter_context(tc.tile_pool(name="tree", bufs=32))
        r_pool = ctx.enter_context(tc.tile_pool(name="r", bufs=3))
        ot_pool = ctx.enter_context(tc.tile_pool(name="ot", bufs=3))
        psum_s = ctx.enter_context(tc.tile_pool(name="psum_s", bufs=2, space="PSUM"))
        psum_out = ctx.enter_context(tc.tile_pool(name="psum_out", bufs=2, space="PSUM"))
        psum_misc = ctx.enter_context(tc.tile_pool(name="psum_misc", bufs=2, space="PSUM"))

        ident = const.tile([P, P], F32)
        make_identity(nc, ident)
        ones_col = const.tile([P, 1], BF16)   # lhsT for partition-sum matmuls
        nc.vector.memset(ones_col, 1.0)
        ones_row = const.tile([1, P], BF16)   # lhsT for broadcast outer-product
        nc.vector.memset(ones_row, 1.0)

        tiles = [{"qt": [], "kt": [], "vn": []} for _ in range(BPC)]

        def prologue_steps(b):
            """24 closures: (kt, qt, v) x 8 groups of 4 n-chunks each."""
            steps = []
            for t in range(NQC):
                for src_ext, tag in ((k_ext, "kt"), (q_ext, "qt")):
                    def tstep(t=t, src_ext=src_ext, tag=tag):
                        ps = psum_misc.tile([P, QC], F32, tag="misc")
                        for j in range(4):
                            nchunk = t * 4 + j
                            natt = nat.tile([P, P], F32, tag="nat")
                            nc.sync.dma_start(
                                out=natt,
                                in_=src_ext[b, nchunk * P:(nchunk + 1) * P, :],
                            )
                            nc.tensor.transpose(ps[:, j * P:(j + 1) * P], natt, ident)
                        pool = kt_pool if tag == "kt" else qt_pool
                        dst = pool.tile([P, QC], BF16, tag=tag)
                        nc.vector.tensor_copy(out=dst, in_=ps)  # f32 -> bf16
                        tiles[b][tag].append(dst)
                    steps.append(tstep)

                def vstep(t=t):
                    vst = stage.tile([P, QC], F32, tag="vst")
                    for j in range(4):
                        nchunk = t * 4 + j
                        nc.sync.dma_start(
                            out=vst[:, j * P:(j + 1) * P],
                            in_=v_ext[b, nchunk * P:(nchunk + 1) * P, :],
                        )
                    vt = vn_pool.tile([P, QC], BF16, tag="vn")
                    nc.vector.tensor_copy(out=vt, in_=vst)
                    tiles[b]["vn"].append(vt)
                steps.append(vstep)
            return steps

        for s in prologue_steps(0):
            s()

        pending_epi = [None]  # deferred epilogue from the previous chunk

        def chunk(b, qc, deferred):
            """Emit one q chunk; run prev chunk's epilogue mid-way; defer own."""
            kt_t, qt_t, vn_t = tiles[b]["kt"], tiles[b]["qt"], tiles[b]["vn"]

            def kt_slice(kc):
                return kt_t[kc // 4][:, (kc % 4) * P:(kc % 4 + 1) * P]

            def vn_slice(kc):
                return vn_t[kc // 4][:, (kc % 4) * P:(kc % 4 + 1) * P]

            rhs_q = qt_t[qc]
            pout = psum_out.tile([P, QC], F32, tag="out")
            pt_slices = [None] * NKC
            leaves = []

            for g in range(NG):
                ps = psum_s.tile([P, KG * QC], F32, tag="s")
                for j in range(KG):
                    kc = g * KG + j
                    nc.tensor.matmul(
                        ps[:, j * QC:(j + 1) * QC],
                        lhsT=kt_slice(kc), rhs=rhs_q,
                        start=True, stop=True,
                    )
                pt = pt_pool.tile([P, KG * QC], BF16, tag="pt")
                nc.scalar.activation(
                    out=pt, in_=ps,
                    func=mybir.ActivationFunctionType.Exp, scale=SCALE,
                )
                for j in range(KG):
                    pt_slices[g * KG + j] = pt[:, j * QC:(j + 1) * QC]
                if g >= 1:
                    for kc in range((g - 1) * KG, g * KG):
                        nc.tensor.matmul(
                            pout, lhsT=vn_slice(kc), rhs=pt_slices[kc],
                            start=(kc == 0), stop=(kc == NKC - 1),
                        )
                if g * KG >= PE_KC:
                    t = tree_pool.tile([P, QC], BF16, tag="tree")
                    nc.gpsimd.tensor_add(
                        out=t, in0=pt[:, 0:QC], in1=pt[:, QC:2 * QC]
                    )
                    leaves.append(t)
                # previous chunk's denominator/normalize runs here, once its
                # DVE-side inputs have had time to land — keeps PE dense
                if g == 3 and pending_epi[0] is not None:
                    pending_epi[0]()
                    pending_epi[0] = None
                # spread the next batch's prologue across this batch's chunks
                if deferred and g in (6, 10, 14):
                    deferred.pop(0)()
            for kc in range((NG - 1) * KG, NKC):
                nc.tensor.matmul(
                    pout, lhsT=vn_slice(kc), rhs=pt_slices[kc],
                    start=(kc == 0), stop=(kc == NKC - 1),
                )

            def epilogue():
                # PE partition-sum for kc < PE_KC, plus folded DVE tree
                psum_d = psum_misc.tile([1, QC], F32, tag="misc")
                for kc in range(PE_KC):
                    nc.tensor.matmul(
                        psum_d, lhsT=ones_col, rhs=pt_slices[kc],
                        start=(kc == 0), stop=False, skip_group_check=True,
                    )
                cur = leaves
                while len(cur) > 1:
                    nxt = []
                    for i in range(0, len(cur) - 1, 2):
                        t = tree_pool.tile([P, QC], BF16, tag="tree")
                        nc.vector.tensor_add(out=t, in0=cur[i], in1=cur[i + 1])
                        nxt.append(t)
                    if len(cur) % 2:
                        nxt.append(cur[-1])
                    cur = nxt
                nc.tensor.matmul(
                    psum_d, lhsT=ones_col, rhs=cur[0],
                    start=False, stop=True, skip_group_check=True,
                )
                rec1 = r_pool.tile([1, QC], F32, tag="rec1")
                nc.vector.reciprocal(rec1, psum_d)
                rec1b = r_pool.tile([1, QC], BF16, tag="rec1b")
                nc.vector.tensor_copy(out=rec1b, in_=rec1)
                psum_r = psum_misc.tile([P, QC], F32, tag="misc")
                nc.tensor.matmul(
                    psum_r, lhsT=ones_row, rhs=rec1b, start=True, stop=True
                )
                rbc = r_pool.tile([P, QC], F32, tag="rbc")
                nc.vector.tensor_copy(out=rbc, in_=psum_r)
                ot = ot_pool.tile([P, QC], F32, tag="ot")
                nc.vector.tensor_mul(out=ot, in0=pout, in1=rbc)
                nc.sync.dma_start(
                    out=out_ext[b, :, qc * QC:(qc + 1) * QC], in_=ot
                )

            # the DVE tree-sum above references `leaves` built in this chunk;
            # emit the whole epilogue during the NEXT chunk
            pending_epi[0] = epilogue

        for b in range(BPC):
            deferred = prologue_steps(b + 1) if b + 1 < BPC else []
            for qc in range(NQC):
                chunk(b, qc, deferred)
            assert not deferred, len(deferred)
        pending_epi[0]()

    return nc


_NC_CACHE = None


def _get_nc():
    global _NC_CACHE
    if _NC_CACHE is None:
        nc = _build()
        _split_waits(nc)
        _NC_CACHE = nc
    return _NC_CACHE


LAST_RESULT = None


def kernel(query, key, value):
    q = np.ascontiguousarray(np.asarray(query), dtype=np.float32)
    k = np.ascontiguousarray(np.asarray(key), dtype=np.float32)
    v = np.ascontiguousarray(np.asarray(value), dtype=np.float32)
    assert q.shape == (B, N, D), q.shape

    nc = _get_nc()
    in_maps = []
    for c in range(NCORES):
        sl = slice(c * BPC, (c + 1) * BPC)
        in_maps.append({
            "query": np.ascontiguousarray(q[sl]),
            "key": np.ascontiguousarray(k[sl]),
            "value": np.ascontiguousarray(v[sl]),
        })

    global LAST_RESULT
    res = run_bass_kernel_spmd(nc, in_maps, core_ids=list(range(NCORES)))
    LAST_RESULT = res

    parts = []
    for c in range(NCORES):
        o = np.asarray(res.results[c]["out"])  # [BPC, D, N]
        parts.append(o.transpose(0, 2, 1))     # -> [BPC, N, D]
    return np.ascontiguousarray(np.concatenate(parts, axis=0), dtype=np.float32)


if __name__ == "__main__":
    nc = _get_nc()
    print("built ok")
